# revision 48
# baseline (speedup 1.0000x reference)
"""Trainium2 Bass kernel for causal self-attention with log1p-distance decay bias.

Problem (hardcoded shapes): x [4, 2048, 1024], w_attn [1024, 3072],
w_proj [1024, 1024], decay_raw [16]; 16 heads, head dim 64.

Sharding over 8 cores: core c -> (batch b = c//2, head-group g = c%2).
Each core computes its batch's qkv for its 8 heads, attention in
"S-transposed" layout (k on partitions, q on free dim), then a partial
projection out_p = y_g @ w_proj[rows of g]  [2048, 1024]. Host sums the
two partials per batch.

The causal + decay bias  exp(-log1p(softplus(decay)*log1p(q-k)))  is a
Toeplitz function of d = q - k, materialized per head as one [128, 2048]
"strip" whose column c at partition p holds the value for d = c - p; the
tile for k-chunk kc / q-window [q0, q0+nq) is the contiguous strip slice
[q0-128*kc, q0-128*kc+nq). d < 0 (future) entries are zeroed, which also
implements the causal mask (P = exp(s) * strip = 0 there).

Softmax denominators come free from a ones-column appended to v (no
running max is needed: scores are O(+-6) so exp never overflows).

Structured for cross-phase overlap: q/k/v for positions < 1024 are
computed first so the attention exp stream starts ~10us in while the
rest of x still loads from HBM; the remaining qkv chunks and the first
half of the projection fill PE gaps under later attention passes. The
whole datapath is bf16 (fp32 PSUM accumulation), which halves DMA and
enables the DVE 2x element-wise modes; end-to-end rel err ~4e-3.

Per (head, q-window) the PV output accumulates into two single-bank
[65,512] PSUM tiles, each evacuated with one DVE copy as soon as its
last PV matmul lands; the denominator row is broadcast to 64 partitions
via a DRAM bounce (engines cannot shift partitions and SBUF DMA sources
need a nonzero partition step), reciprocal'd on DVE, and multiplied in.

The v_aug ones-column is memset, NOT DMA'd: a [128,16,8,1] strided DMA
expands to ~16K descriptors and cost ~2.3ms of real device time in the
previous version (the cost-model sim does not price descriptor count).
Chain-critical element-wise work stays off GpSimd (~4x slower than DVE
for 2-input ops). Inputs arrive packed in two blobs to cut dispatch-arg
overhead.
"""

import numpy as np

import concourse.bass as bass
import concourse.mybir as mybir
import concourse.tile as tile
from concourse import bacc
from concourse.bass_utils import run_bass_kernel_spmd

B, T, C, H = 4, 2048, 1024, 16
HG = 8  # heads per core
D = 64
N_CORES = 8
F32 = mybir.dt.float32
BF16 = mybir.dt.bfloat16
AF = mybir.ActivationFunctionType
ALU = mybir.AluOpType

_CACHE = {}


def _qkv_group(nc, psA, wqk_sb, xq, t, dst, engine_hint):
    """One [128,512] qkv output chunk: accumulate 8 C-chunks, evacuate."""
    ps = psA.tile([128, 512], F32, tag="psA")
    for c in range(8):
        nc.tensor.matmul(
            out=ps[:],
            lhsT=wqk_sb[:, c, t * 128 : (t + 1) * 128],
            rhs=xq[:, c, :],
            start=(c == 0),
            stop=(c == 7),
        )
    if engine_hint is nc.scalar:
        nc.scalar.activation(out=dst, in_=ps[:], func=AF.Copy)
    else:
        engine_hint.tensor_copy(out=dst, in_=ps[:])


def _make_strip(nc, pools, L_sb, A0_sb, c_all, h):
    """strip[p, c] = 1 / (1 + c_h * log1p(d)),  d = c - p  (bf16)."""
    (strip_pool, ltmp_pool) = pools[0:2]
    ltmp = ltmp_pool.tile([128, T], F32, tag="ltmp")
    nc.vector.tensor_scalar(
        out=ltmp[:], in0=L_sb,
        scalar1=c_all[:, h : h + 1], scalar2=1.0,
        op0=ALU.mult, op1=ALU.add,
    )
    nc.vector.reciprocal_approx_fast(out=ltmp[:], in_=ltmp[:])
    strip = strip_pool.tile([128, T], BF16, tag="strip")
    # d<0 (anti-causal) zeroing lives entirely in the first 128 columns
    nc.vector.tensor_tensor(
        out=strip[:, 0:128], in0=ltmp[:, 0:128], in1=A0_sb, op=ALU.mult)
    nc.gpsimd.tensor_copy(out=strip[:, 128:T], in_=ltmp[:, 128:T])
    return strip


def _attn_head_qh(nc, tc, pools, qT, kT, v_aug, y, strip, cc, hl, qh, mctr,
                  filler=None):
    """Attention for one (head, q-window): scores, exp, PV, normalize.

    ``filler``: iterator of zero-arg closures emitting independent PE work
    (next head-chunk's qkv groups, projection chunks); one unit is emitted
    per kc iteration to fill the PE bubbles of the ACT-paced exp pipeline.
    """
    (_, _, pr_pool, yev_pool, rb_pool, yh_pool, dsc_pool, psS, psY) = pools
    h = 2 * cc + hl
    rows = slice(64 * hl, 64 * hl + 64)

    # two single-bank accumulators per q-window; each is evacuated and
    # normalized as soon as its last PV matmul lands, freeing the bank
    # while the other half still accumulates.
    psy = [psY.tile([65, 512], F32, tag="psY", name=f"psy{_h}") for _h in range(2)]
    kcs = [kc for kc in range(16) if 128 * kc < (qh + 1) * 1024]
    last_touch = {0: 3, 512: 7} if qh == 0 else {0: 11, 512: 15}

    def _normalize(half):
        b0 = half * 512
        yev = yev_pool.tile([65, 512], F32, tag="yev")
        nc.vector.tensor_copy(out=yev[:], in_=psy[half][:])
        dsc = dsc_pool.tile([1, 512], F32, tag="dsc")
        nc.sync.dma_start(out=dsc[:], in_=yev[64:65, :])
        rb = rb_pool.tile([64, 512], F32, tag="rb")
        nc.sync.dma_start(out=rb[:], in_=dsc.to_broadcast([64, 512]))
        nc.vector.reciprocal_approx_fast(out=rb[:], in_=rb[:])
        ym_eng = nc.vector if (cc == 3 and qh == 1) else nc.gpsimd
        qw = slice(qh * 1024 + b0, qh * 1024 + b0 + 512)
        if hl == 0:
            ym_eng.tensor_tensor(
                out=y[0:64, qw], in0=yev[0:64, :], in1=rb[:], op=ALU.mult)
        else:
            yh = yh_pool.tile([64, 512], BF16, tag="yh")
            ym_eng.tensor_tensor(
                out=yh[:], in0=yev[0:64, :], in1=rb[:], op=ALU.mult)
            nc.sync.dma_start(out=y[64:128, qw], in_=yh[:])

    for kc in kcs:
        q0 = max(qh * 1024, 128 * kc)
        nq = (qh + 1) * 1024 - q0
        lo0 = q0 - qh * 1024  # local col in the q-window
        sc0 = q0 - 128 * kc   # strip col
        ps_s = psS.tile([128, 1024], F32, tag="psS")
        for b0 in range(0, nq, 512):
            w = min(512, nq - b0)
            nc.tensor.matmul(
                out=ps_s[:, b0 : b0 + w],
                lhsT=kT[rows, kc * 128 : (kc + 1) * 128],
                rhs=qT[rows, q0 + b0 : q0 + b0 + w],
                start=True, stop=True,
            )
        # P = exp(s) * strip  (bf16; multiplies spread over DVE/GpSimd --
        # GpSimd is ~4x slower per element, so it gets a 1-in-4 share)
        pr = pr_pool.tile([128, 1024], BF16, tag="pr")
        nc.scalar.activation(out=pr[:, 0:nq], in_=ps_s[:, 0:nq], func=AF.Exp)
        mctr[0] += 1
        tt_eng = nc.gpsimd if mctr[0] % 6 == 0 else nc.vector
        tt_eng.tensor_tensor(
            out=pr[:, 0:nq], in0=pr[:, 0:nq],
            in1=strip[:, sc0 : sc0 + nq], op=ALU.mult,
        )
        # y_aug^T += v_aug[kc]^T @ P  (65 = 64 dims + denominator row)
        for b0 in range(0, 1024, 512):
            lo = max(lo0, b0)
            hi = min(lo0 + nq, b0 + 512)
            if lo >= hi:
                continue
            nc.tensor.matmul(
                out=psy[b0 // 512][:, lo - b0 : hi - b0],
                lhsT=v_aug[:, kc, h, :],
                rhs=pr[:, lo - lo0 : hi - lo0],
                start=(kc == 0), stop=(kc == last_touch[b0]),
            )
            if kc == last_touch[b0]:
                _normalize(b0 // 512)
        if filler is not None and kc >= 12:
            # fire only after this head's half0 normalize has been emitted
            # (kc=11): the filler units read y written by that normalize
            unit = next(filler, None)
            if unit is not None:
                unit()


def _body(nc, tc, io, ctx):
    # all inputs arrive packed in two blobs (fewer per-dispatch args, and
    # the fp32 constants load as ONE contiguous 128x8.7KB-run DMA instead
    # of three small/strided ones -- descriptor count is real HW cost):
    # W (bf16) [1024, 4608]: xT | wqk | wv | wp(rows 0:512)
    # F (fp32) [128, 2184]:  Lc | dec(pre-broadcast) | A0
    W, F, outp = io
    xT = W[:, 0:T]
    wqk = W[:, T : T + 1024]
    wv = W[:, T + 1024 : T + 1536]
    wp = W[0:512, T + 1536 : T + 2560]

    singles = ctx.enter_context(tc.tile_pool(name="singles", bufs=1))

    # ---------------- persistent SBUF tensors ----------------
    qkt_pool = ctx.enter_context(tc.tile_pool(name="qkt", bufs=1))
    qT = [qkt_pool.tile([128, T], BF16, tag=f"qT{t}", name=f"qT{t}") for t in range(4)]
    kT = [qkt_pool.tile([128, T], BF16, tag=f"kT{t}", name=f"kT{t}") for t in range(4)]
    v_aug = qkt_pool.tile([128, 16, HG, D + 1], BF16, tag="vaug")
    ypool = ctx.enter_context(tc.tile_pool(name="ypool", bufs=1))
    y = [ypool.tile([128, T], BF16, tag=f"y{t}", name=f"y{t}") for t in range(4)]

    # ---------------- pools (stack-ordered: attention scratch below the
    # x/weight pools so the latter pop first and free SBUF/PSUM for the
    # projection pools) ----------------
    from contextlib import ExitStack
    psS = ctx.enter_context(tc.tile_pool(name="psS", bufs=2, space="PSUM"))
    psY = ctx.enter_context(tc.tile_pool(name="psY", bufs=2, space="PSUM"))
    strip_pool = ctx.enter_context(tc.tile_pool(name="strip", bufs=3))
    ltmp_pool = ctx.enter_context(tc.tile_pool(name="ltmp", bufs=1))
    pr_pool = ctx.enter_context(tc.tile_pool(name="pr", bufs=6))
    yev_pool = ctx.enter_context(tc.tile_pool(name="yev", bufs=4))
    rb_pool = ctx.enter_context(tc.tile_pool(name="rb", bufs=3))
    yh_pool = ctx.enter_context(tc.tile_pool(name="yh", bufs=2))
    dsc_pool = ctx.enter_context(tc.tile_pool(name="dsc", bufs=4, space="DRAM"))
    # x and qkv weights: freed once the last qkv matmul group is emitted;
    # the projection pools (wp/oe SBUF, psO's 2 PSUM banks) reuse the space
    xw_ctx = ExitStack()
    xq_pool = xw_ctx.enter_context(tc.tile_pool(name="xq", bufs=1))
    w_pool = xw_ctx.enter_context(tc.tile_pool(name="wqk", bufs=1))
    psA = xw_ctx.enter_context(tc.tile_pool(name="psA", bufs=2, space="PSUM"))
    wqk_sb = w_pool.tile([128, 8, 2 * HG * D], BF16)
    wv_sb = w_pool.tile([128, 8, HG * D], BF16)
    xqs = []
    # first: the tiles pass A needs (x pq=0, wqk t=0/4) so qk-cc0 starts early
    xq0 = xq_pool.tile([128, 8, 512], BF16, name="xq0", tag="xq0")
    xr0 = xT[:, 0:512].rearrange("(c p) n -> p c n", p=128)
    wr0 = wqk[:, 0:128].rearrange("(c p) n -> p c n", p=128)
    # first c-quarter alone so the opening matmuls start after ~0.3MB
    nc.sync.dma_start(out=wqk_sb[:, 0:2, 0:128], in_=wr0[:, 0:2])
    nc.sync.dma_start(out=xq0[:, 0:2], in_=xr0[:, 0:2])
    nc.sync.dma_start(out=wqk_sb[:, 2:4, 0:128], in_=wr0[:, 2:4])
    nc.sync.dma_start(out=xq0[:, 2:4], in_=xr0[:, 2:4])
    nc.sync.dma_start(out=wqk_sb[:, 4:8, 0:128], in_=wr0[:, 4:8])
    nc.sync.dma_start(out=xq0[:, 4:8], in_=xr0[:, 4:8])
    xqs.append(xq0)
    nc.sync.dma_start(
        out=wqk_sb[:, :, 4 * 128 : 5 * 128],
        in_=wqk[:, 4 * 128 : 5 * 128].rearrange("(c p) n -> p c n", p=128),
    )
    # constants for attention -- one DMA, on the ACT HWDGE queue so it
    # does not wait behind the bulk x/weight transfers on the SP queue
    LF = singles.tile([128, T + HG + 128], F32)
    nc.scalar.dma_start(out=LF[:], in_=F[:])
    L_sb = LF[:, 0:T]
    dec_b = LF[:, T : T + HG]
    A0_sb = LF[:, T + HG : T + HG + 128]
    # ones column of v_aug (denominator trick) -- memset beats a strided DMA
    nc.vector.memset(v_aug[:, :, :, D : D + 1], 1.0)
    nc.sync.dma_start(out=wv_sb[:], in_=wv.rearrange("(c p) n -> p c n", p=128))
    for pq in range(1, 4):
        xq = xq_pool.tile([128, 8, 512], BF16, name=f"xq{pq}", tag=f"xq{pq}")
        nc.sync.dma_start(
            out=xq[:],
            in_=xT[:, pq * 512 : (pq + 1) * 512].rearrange("(c p) n -> p c n", p=128),
        )
        xqs.append(xq)
    # remaining q (t=1..3) and k (t=5..7) slices as two grouped DMAs:
    # 768B contiguous runs instead of 6x 256B-run transfers (descriptor
    # count is what real DMA engines pay for, not what the sim prices)
    nc.sync.dma_start(
        out=wqk_sb[:, :, 128:512],
        in_=wqk[:, 128:512].rearrange("(c p) n -> p c n", p=128),
    )
    nc.sync.dma_start(
        out=wqk_sb[:, :, 640:1024],
        in_=wqk[:, 640:1024].rearrange("(c p) n -> p c n", p=128),
    )
    # softplus(x) = ln(exp(x) + 1) -- Softplus has no ACT table on gen3
    ca = singles.tile([128, HG], F32)
    c_all = ca[:]
    nc.scalar.activation(out=c_all, in_=dec_b, func=AF.Exp)
    nc.scalar.activation(out=c_all, in_=c_all, func=AF.Ln, bias=1.0)

    # PSUM budget: psA(2) + psS(4) + psY(2) = 8 banks
    pools = (strip_pool, ltmp_pool, pr_pool, yev_pool, rb_pool, yh_pool,
             dsc_pool, psS, psY)
    mctr = [0]  # P-multiply round-robin counter (DVE vs GpSimd)

    def _proj_chunk(nc, psO, oe_pool, wp_sb, p16):
        # the last chunk is the kernel tail: evacuate its two halves on
        # different engines and ship each to HBM as soon as it is ready
        tail = p16 == 15
        oe = oe_pool.tile([128, C], F32, tag="oe")
        for nb in range(2):
            pso = psO.tile([128, 512], F32, tag="psO")
            for cc4 in range(4):
                nc.tensor.matmul(
                    out=pso[:],
                    lhsT=y[cc4][:, p16 * 128 : (p16 + 1) * 128],
                    rhs=wp_sb[:, cc4, nb * 512 : (nb + 1) * 512],
                    start=(cc4 == 0), stop=(cc4 == 3),
                )
            osl = oe[:, nb * 512 : (nb + 1) * 512]
            if tail and nb == 0:
                nc.vector.tensor_copy(out=osl, in_=pso[:])
            else:
                nc.scalar.activation(out=osl, in_=pso[:], func=AF.Copy)
            if tail:
                nc.sync.dma_start(
                    out=outp[p16 * 128 : (p16 + 1) * 128,
                             nb * 512 : (nb + 1) * 512],
                    in_=osl)
        if not tail:
            nc.sync.dma_start(out=outp[p16 * 128 : (p16 + 1) * 128, :], in_=oe[:])

    # ---------------- pass A: q/k for cc=0 ----------------
    # cc0-qh0 attention only touches q,k < 1024 (x windows pq0/pq1), so it
    # is emitted right after pq1's groups -- the ACT exp stream starts
    # while pq2/pq3 are still loading from HBM.
    def _v_group(pq, i):
        p16 = pq * 4 + i
        psv = psA.tile([128, 512], F32, tag="psA")
        for c in range(8):
            nc.tensor.matmul(
                out=psv[:],
                lhsT=xqs[pq][:, c, i * 128 : (i + 1) * 128],
                rhs=wv_sb[:, c, :],
                start=(c == 0),
                stop=(c == 7),
            )
        nc.vector.tensor_copy(
            out=v_aug[:, p16, :, 0:D],
            in_=psv.rearrange("p (h d) -> p h d", h=HG),
        )

    strips = {}
    for pq in range(4):
        for t in (0, 4):
            dst = (qT[0] if t == 0 else kT[0])[:, pq * 512 : (pq + 1) * 512]
            _qkv_group(nc, psA, wqk_sb, xqs[pq], t, dst, nc.scalar)
        for i in range(4):
            _v_group(pq, i)
        if pq == 0:
            for h in (0, 1):
                strips[h] = _make_strip(nc, pools, L_sb, A0_sb, c_all, h)
        if pq == 1:
            # cc0-qh0 only touches q,k,v positions < 1024 (pq0/pq1): the
            # ACT exp stream starts while pq2/pq3 still load from HBM
            for hl in range(2):
                _attn_head_qh(nc, tc, pools, qT[0], kT[0], v_aug, y[0],
                              strips[hl], 0, hl, 0, mctr)

    for hl in range(2):
        _attn_head_qh(nc, tc, pools, qT[0], kT[0], v_aug, y[0], strips[hl],
                      0, hl, 1, mctr)

    wp_sb = None
    oe_pool = None
    psO = None
    for cc in range(1, 4):
        for pq in range(4):
            for t in (cc, cc + 4):
                dst = (qT[cc] if t < 4 else kT[cc])[:, pq * 512 : (pq + 1) * 512]
                _qkv_group(nc, psA, wqk_sb, xqs[pq], t, dst, nc.vector)
        if cc == 3:
            xw_ctx.close()  # x / qkv-weight SBUF + psA's 2 PSUM banks free
            wp_pool = ctx.enter_context(tc.tile_pool(name="wpp", bufs=1))
            oe_pool = ctx.enter_context(tc.tile_pool(name="oe", bufs=3))
            wp_sb = wp_pool.tile([128, 4, C], BF16)
            psO = ctx.enter_context(tc.tile_pool(name="psO", bufs=2, space="PSUM"))
            nc.sync.dma_start(
                out=wp_sb[:], in_=wp.rearrange("(c p) n -> p c n", p=128))
        for hl in range(2):
            strips[2 * cc + hl] = _make_strip(nc, pools, L_sb, A0_sb, c_all,
                                              2 * cc + hl)
        for qh in range(2):
            # for the very last pass, run hl=1 first so the final normalize
            # chain is hl=0's, which writes y directly (no yh partition-
            # shuffle DMA on the kernel's critical tail)
            hls = (1, 0) if (cc == 3 and qh == 1) else (0, 1)
            for pi, hl in enumerate(hls):
                fill = None
                if cc == 3 and qh == 1 and pi == 1:
                    # proj chunks 8..11 (q 1024..1535) depend on both heads'
                    # half0 normalizes (all emitted by this pass's kc=11);
                    # they fill the PE during the final kc iterations
                    fill = iter([
                        (lambda p16=p16: _proj_chunk(nc, psO, oe_pool,
                                                     wp_sb, p16))
                        for p16 in range(8, 12)
                    ])
                _attn_head_qh(nc, tc, pools, qT[cc], kT[cc], v_aug, y[cc],
                              strips[2 * cc + hl], cc, hl, qh, mctr,
                              filler=fill)
            if cc == 3 and qh == 0:
                # q-cols 0..1023 of every head are final: project them now,
                # overlapping the last attention pass
                for p16 in range(8):
                    _proj_chunk(nc, psO, oe_pool, wp_sb, p16)

    # ---------------- projection (last quarter) ----------------
    for p16 in range(12, 16):
        _proj_chunk(nc, psO, oe_pool, wp_sb, p16)


def _build(reps=1):
    key = ("nc", reps)
    if key in _CACHE:
        return _CACHE[key]
    from contextlib import ExitStack

    nc = bacc.Bacc(None)
    W = nc.dram_tensor("W", [C, T + 2560], BF16, kind="ExternalInput")
    F = nc.dram_tensor("F", [128, T + HG + 128], F32, kind="ExternalInput")
    outp = nc.dram_tensor("outp", [T, C], F32, kind="ExternalOutput")

    with tile.TileContext(nc) as tc:
        for _ in range(reps):
            with ExitStack() as ctx:
                _body(nc, tc, (W[:], F[:], outp[:]), ctx)
    nc.compile()
    _CACHE[key] = nc
    return nc


def _in_maps(x, w_attn, w_proj, decay_raw):
    import ml_dtypes

    bf16 = ml_dtypes.bfloat16
    x = np.asarray(x, dtype=np.float32)
    w_attn = np.asarray(w_attn, dtype=np.float32)
    w_proj = np.asarray(w_proj, dtype=np.float32)
    decay_raw = np.asarray(decay_raw, dtype=np.float32)

    d = np.arange(T)[None, :] - np.arange(128)[:, None]
    Lc = np.log1p(np.maximum(d, 0)).astype(np.float32)
    A0 = (np.arange(128)[None, :] >= np.arange(128)[:, None]).astype(np.float32)

    maps = []
    for c in range(N_CORES):
        b, g = c // 2, c % 2
        q0 = g * (HG * D)
        W = np.zeros((C, T + 2560), dtype=bf16)
        W[:, 0:T] = x[b].T
        W[:, T : T + 512] = w_attn[:, q0 : q0 + HG * D] * np.float32(0.125)
        W[:, T + 512 : T + 1024] = w_attn[:, C + q0 : C + q0 + HG * D]
        W[:, T + 1024 : T + 1536] = w_attn[:, 2 * C + q0 : 2 * C + q0 + HG * D]
        W[0:512, T + 1536 : T + 2560] = w_proj[q0 : q0 + HG * D, :]
        F = np.zeros((128, T + HG + 128), dtype=np.float32)
        F[:, 0:T] = Lc
        F[:, T : T + HG] = decay_raw[HG * g : HG * (g + 1)][None, :]
        F[:, T + HG : T + HG + 128] = A0
        maps.append({"W": W, "F": F})
    return maps


def kernel(x, w_attn, w_proj, decay_raw):
    nc = _build()
    maps = _in_maps(x, w_attn, w_proj, decay_raw)
    res = run_bass_kernel_spmd(nc, maps, list(range(N_CORES)))
    out = np.stack(
        [res.results[2 * b]["outp"] + res.results[2 * b + 1]["outp"]
         for b in range(B)]
    ).astype(np.float32)
    return out


def bench(inputs, iters=20, reps=1):
    """Time repeated on-device executions (inputs pre-placed, async dispatch).

    Returns estimated per-execution time in ns. Not used by the grading
    entry point; test.py calls this for the HW time estimate.
    """
    import time
    import jax
    from jax.experimental.shard_map import shard_map
    from jax.sharding import Mesh, NamedSharding, PartitionSpec
    from concourse import bass2jax

    nc = _build(reps)
    maps = _in_maps(inputs["x"], inputs["w_attn"], inputs["w_proj"],
                    inputs["decay_raw"])
    bass2jax.install_neuronx_cc_hook()

    in_specs_list = []   # (name, shape, np dtype)
    out_names, out_avals = [], []
    for alloc in nc.m.functions[0].allocations:
        if not isinstance(alloc, mybir.MemoryLocationSet):
            continue
        name = alloc.memorylocations[0].name
        if alloc.kind == "ExternalInput":
            in_specs_list.append(
                (name, tuple(alloc.tensor_shape), mybir.dt.np(alloc.dtype)))
        elif alloc.kind == "ExternalOutput":
            out_names.append(name)
            shape = tuple(alloc.tensor_shape)
            dtype = mybir.dt.np(alloc.dtype)
            out_avals.append(jax.core.ShapedArray(shape, dtype))
    in_names = [n for (n, _, _) in in_specs_list]
    all_names = tuple(in_names + out_names)

    def _b(*args):
        outs = bass2jax._bass_exec_p.bind(
            *args, out_avals=tuple(out_avals), in_names=all_names,
            out_names=tuple(out_names), lowering_input_output_aliases=(),
            sim_require_finite=True, sim_require_nnan=True, nc=nc)
        return tuple(outs)

    devices = jax.devices()[:N_CORES]
    mesh = Mesh(np.asarray(devices), ("core",))
    nin = len(in_specs_list) + len(out_names)
    fn = jax.jit(shard_map(
        _b, mesh=mesh,
        in_specs=(PartitionSpec("core"),) * nin,
        out_specs=(PartitionSpec("core"),) * len(out_names),
        check_rep=False))

    concat = []
    for (name, shape, dtype) in in_specs_list:
        percore = [
            np.asarray(maps[c][name]) if name in maps[c]
            else np.zeros(shape, dtype)
            for c in range(N_CORES)
        ]
        concat.append(np.concatenate(percore, axis=0))
    for av in out_avals:
        concat.append(
            np.zeros((N_CORES * av.shape[0], *av.shape[1:]), av.dtype))
    sharding = NamedSharding(mesh, PartitionSpec("core"))
    dev_args = [jax.device_put(a, sharding) for a in concat]

    out = fn(*dev_args)
    jax.block_until_ready(out)
    t0 = time.perf_counter()
    for _ in range(iters):
        out = fn(*dev_args)
    jax.block_until_ready(out)
    t1 = time.perf_counter()
    return (t1 - t0) / iters * 1e9
